# revision 11
# baseline (speedup 1.0000x reference)
"""ClusterLoss Trainium2 Bass kernel (8-core SPMD).

Problem (P=256 ids, K=16 imgs/id, D=2048):
  centers   = mean over K of features.reshape(P,K,D)           [P, D]
  intra_max = max_k ||f[p,k]-c[p]||  (clipped sqrt)            [P]
  inter_min = min_{j!=i} ||c[i]-c[j]||                         [P]
  loss      = mean(relu(intra_max - inter_min + 10))

Sharding: class dim P across 8 cores (32 classes = 512 rows each).
Per core, over 4 column-chunks (512 f32) x 4 row-groups (128 rows):
  - PE: class-sum matmuls (indicator lhsT) -> centers [32,512] per chunk
  - DMA: broadcast-replicate centers back to [128,512] row alignment
  - DVE: diff = f - rep ; ACT: Square+accumulate -> intra d2
  - centers are PE-transposed and AllGathered per chunk (overlapped),
    each core then computes its [32,256] block of the center Gram to
    get inter_min; cc (center norms) ship with the gather.
Outputs per core: intra [1,32], inter [1,32], loss partial [1,1].
Host: concat + sum/256.
"""

import numpy as np

import concourse.bacc as bacc
import concourse.tile as tile
from concourse import mybir
from concourse.bass_utils import run_bass_kernel_spmd
from contextlib import ExitStack

F32 = mybir.dt.float32
AF = mybir.ActivationFunctionType
ALU = mybir.AluOpType

N_CORES = 8
P = 256
K = 16
D = 2048
PL = P // N_CORES          # 32 local classes
R = PL * K                 # 512 local rows
NC_CHUNK = 4               # column chunks
CW = D // NC_CHUNK         # 512 cols per chunk
NR = 4                     # row groups
RW = R // NR               # 128 rows per group
GR = PL // NR              # 8 classes per row group
MARGIN = 10.0
EPS = 1e-12
BIG = 1e30

_CACHE = {}


def _build():
    nc = bacc.Bacc("TRN2", target_bir_lowering=False, debug=False,
                   num_devices=N_CORES)

    feat = nc.dram_tensor("features", [R, D], F32, kind="ExternalInput").ap()
    ind32 = nc.dram_tensor("ind32", [RW, NR * PL], F32,
                           kind="ExternalInput").ap()
    ident = nc.dram_tensor("ident", [128, 128], F32, kind="ExternalInput").ap()
    maskb = nc.dram_tensor("maskb", [PL, P], F32, kind="ExternalInput").ap()
    ones4 = nc.dram_tensor("ones4", [NC_CHUNK, PL], F32,
                           kind="ExternalInput").ap()
    ones32 = nc.dram_tensor("ones32", [PL, 1], F32, kind="ExternalInput").ap()
    coreid = nc.dram_tensor("coreid", [1, 1], F32, kind="ExternalInput").ap()

    intra_o = nc.dram_tensor("intra", [1, PL], F32, kind="ExternalOutput").ap()
    inter_o = nc.dram_tensor("inter", [1, PL], F32, kind="ExternalOutput").ap()
    loss_o = nc.dram_tensor("loss", [1, 1], F32, kind="ExternalOutput").ap()

    core = None  # resolved via maskb input (per-core constant)

    with tile.TileContext(nc) as tc, ExitStack() as ctx:
        consts = ctx.enter_context(tc.tile_pool(name="consts", bufs=1))
        fpool = ctx.enter_context(tc.tile_pool(name="f", bufs=6))
        reppool = ctx.enter_context(tc.tile_pool(name="rep", bufs=4))
        diffpool = ctx.enter_context(tc.tile_pool(name="diff", bufs=3))
        sqpool = ctx.enter_context(tc.tile_pool(name="sq", bufs=2))
        centpool = ctx.enter_context(tc.tile_pool(name="cent", bufs=2))
        ctTpool = ctx.enter_context(tc.tile_pool(name="ctT", bufs=2))
        ctApool = ctx.enter_context(tc.tile_pool(name="ctA", bufs=4))
        small = ctx.enter_context(tc.tile_pool(name="small", bufs=1))
        epi = ctx.enter_context(tc.tile_pool(name="epi", bufs=1))
        ps_cent = ctx.enter_context(
            tc.tile_pool(name="ps_cent", bufs=2, space="PSUM"))
        ps_t = ctx.enter_context(
            tc.tile_pool(name="ps_t", bufs=1, space="PSUM"))
        ps_g = ctx.enter_context(
            tc.tile_pool(name="ps_g", bufs=1, space="PSUM"))
        ps_misc = ctx.enter_context(
            tc.tile_pool(name="ps_misc", bufs=1, space="PSUM"))
        dram = ctx.enter_context(
            tc.tile_pool(name="dram", bufs=1, space="DRAM"))

        # ---- constants ----
        ind32_t = consts.tile([RW, NR * PL], F32)
        nc.sync.dma_start(ind32_t[:], ind32[:])
        ident_t = consts.tile([128, 128], F32)
        nc.sync.dma_start(ident_t[:], ident[:])
        maskb_t = consts.tile([PL, P], F32)
        nc.sync.dma_start(maskb_t[:], maskb[:])
        ones4_t = consts.tile([NC_CHUNK, PL], F32)
        nc.sync.dma_start(ones4_t[:], ones4[:])
        ones32_t = consts.tile([PL, 1], F32)
        nc.sync.dma_start(ones32_t[:], ones32[:])

        # persistent accumulators
        d2cols = small.tile([RW, NR], F32)      # intra d2 sums, col = rowgroup
        cc_loc = small.tile([PL, 1], F32)       # local center norms
        cent_full = small.tile([PL, D], F32)    # all centers (means)

        BF16 = mybir.dt.bfloat16
        SHIP_R = 4 * 128 + 2   # 512 center rows + cc_hi + cc_lo (only c=3)
        ship = dram.tile([NC_CHUNK, SHIP_R, PL], BF16, tag="ship")
        g = dram.tile([N_CORES, NC_CHUNK, SHIP_R, PL], BF16, tag="gath")

        # ---- full-row feature loads: 4 x 1MB contiguous DMAs ----
        f_tiles = []
        for r in range(NR):
            ft = fpool.tile([RW, D], F32, tag="f")
            nc.sync.dma_start(ft[:], feat[r * RW:(r + 1) * RW, :])
            f_tiles.append(ft)

        # ---- centers: per column-chunk accumulate over row groups ----
        for c in range(NC_CHUNK):
            cent_ps = ps_cent.tile([PL, CW], F32, tag="cent")
            for r in range(NR):
                nc.tensor.matmul(cent_ps[:],
                                 lhsT=ind32_t[:, PL * r:PL * (r + 1)],
                                 rhs=f_tiles[r][:, c * CW:(c + 1) * CW],
                                 start=(r == 0), stop=(r == NR - 1))
            # extract + scale to means into the full center tile
            nc.scalar.activation(cent_full[:, c * CW:(c + 1) * CW], cent_ps[:],
                                 AF.Copy, scale=1.0 / K)

            # transpose this chunk for shipping (bf16)
            ctT_ps = ps_t.tile([128, 4 * PL], F32, tag="ctT")
            for s in range(4):
                nc.tensor.transpose(
                    ctT_ps[:, s * PL:(s + 1) * PL],
                    cent_full[:, c * CW + s * 128:c * CW + (s + 1) * 128],
                    ident_t[0:PL, 0:PL])
            ctT_sb = ctTpool.tile([128, 4 * PL], BF16, tag="ctT_sb")
            nc.vector.tensor_copy(ctT_sb[:], ctT_ps[:])
            for s in range(4):
                nc.gpsimd.dma_start(ship[c, s * 128:(s + 1) * 128, :],
                                    ctT_sb[:, s * PL:(s + 1) * PL])

        # local center norms (full D, one pass) + bf16 hi/lo for shipping
        sq_cc = sqpool.tile([PL, D], F32, tag="sqcc")
        nc.scalar.activation(sq_cc[:], cent_full[:], AF.Square,
                             accum_out=cc_loc[:])
        cc_hi = epi.tile([PL, 1], BF16, tag="cchi")
        nc.vector.tensor_copy(cc_hi[:], cc_loc[:])
        cc_lo = epi.tile([PL, 1], BF16, tag="cclo")
        nc.vector.tensor_tensor(cc_lo[:], cc_loc[:], cc_hi[:], ALU.subtract)
        nc.gpsimd.dma_start(ship[3, 4 * 128:4 * 128 + 1, :], cc_hi[:])
        nc.gpsimd.dma_start(ship[3, 4 * 128 + 1:4 * 128 + 2, :], cc_lo[:])

        # one AllGather for everything
        nc.gpsimd.collective_compute(
            "AllGather", ALU.bypass,
            replica_groups=[list(range(N_CORES))],
            ins=[ship[:].opt()], outs=[g[:].opt()])

        # ---- intra: replicate via DRAM bounce (avoids re-reading the
        # same SBUF partitions 16x, which serializes on one port) ----
        cent_d = dram.tile([PL, D], F32, tag="cent_d")
        nc.sync.dma_start(cent_d[:], cent_full[:])
        for r in range(NR):
            rep = reppool.tile([RW, D], F32, tag="rep")
            srcc = cent_d[r * GR:(r + 1) * GR, :]
            eng = nc.scalar if r % 2 == 0 else nc.sync
            eng.dma_start(rep[:], srcc.unsqueeze(1).broadcast_to([GR, K, D]))
            diff = diffpool.tile([RW, D], F32, tag="diff")
            nc.vector.tensor_tensor(diff[:], f_tiles[r][:], rep[:],
                                    ALU.subtract)
            sq = sqpool.tile([RW, D], F32, tag="sq")
            nc.scalar.activation(sq[:], diff[:], AF.Square,
                                 accum_out=d2cols[:, r:r + 1])

        # ---- inter: gram over gathered transposed centers (bf16) ----
        g_ps = ps_g.tile([PL, P], F32, tag="G")
        n_mm = 0
        for c in range(NC_CHUNK):
            for s in range(4):
                ctA = ctApool.tile([128, P], BF16, tag="ctA")
                nc.gpsimd.dma_start(
                    ctA[:].rearrange("p (w i) -> p w i", w=N_CORES),
                    g[:, c, s * 128:(s + 1) * 128, :]
                    .rearrange("w r i -> r w i"))
                lloc = ctApool.tile([128, PL], BF16, tag="lloc")
                nc.gpsimd.dma_start(
                    lloc[:], ship[c, s * 128:(s + 1) * 128, :])
                nc.tensor.matmul(g_ps[:], lhsT=lloc[:], rhs=ctA[:],
                                 start=(n_mm == 0), stop=(n_mm == 15))
                n_mm += 1
        # cc of all cores (hi + lo rows from chunk 3 section)
        cch_sb = small.tile([1, P], BF16)
        ccl_sb = small.tile([1, P], BF16)
        nc.gpsimd.dma_start(cch_sb[:], g[:, 3, 4 * 128:4 * 128 + 1, :])
        nc.gpsimd.dma_start(ccl_sb[:], g[:, 3, 4 * 128 + 1:4 * 128 + 2, :])
        ccp_sb = small.tile([1, P], F32)
        nc.vector.tensor_tensor(ccp_sb[:], cch_sb[:], ccl_sb[:], ALU.add)

        # ccrep [PL, P] = sum over chunks of ccp, broadcast to PL partitions
        ccrep_ps = ps_misc.tile([PL, P], F32, tag="ccrep")
        nc.tensor.matmul(ccrep_ps[:], lhsT=ones4_t[0:1, :], rhs=ccp_sb[:],
                         start=True, stop=True)
        in1c = epi.tile([PL, P], F32)
        nc.vector.tensor_tensor(in1c[:], ccrep_ps[:], maskb_t[:], ALU.add)
        H = epi.tile([PL, P], F32)
        nc.vector.scalar_tensor_tensor(H[:], g_ps[:], -2.0, in1c[:],
                                       ALU.mult, ALU.add)
        m2 = epi.tile([PL, 1], F32)
        nc.vector.tensor_reduce(m2[:], H[:], mybir.AxisListType.X, ALU.min)
        inter2 = epi.tile([PL, 1], F32)
        nc.vector.tensor_tensor(inter2[:], m2[:], cc_loc[:], ALU.add)
        inter_sb = _sqrt_newton(nc, epi, inter2, PL, 1)

        # ---- intra: d2 sums -> per-class max ----
        d2T_ps = ps_misc.tile([NR, RW], F32, tag="d2T")
        nc.tensor.transpose(d2T_ps[:], d2cols[:], ident_t[:])
        d2T = epi.tile([NR, RW], F32)
        nc.vector.tensor_copy(d2T[:], d2T_ps[:])
        dmax = epi.tile([NR, GR], F32)
        nc.vector.tensor_reduce(
            dmax[:], d2T[:].rearrange("r (g k) -> r g k", k=K),
            mybir.AxisListType.X, ALU.max)
        intra_sb = _sqrt_newton(nc, epi, dmax, NR, GR)

        # outputs (view the DRAM side to match on-chip layouts)
        nc.sync.dma_start(
            intra_o[:].rearrange("o (r g) -> (o r) g", r=NR), intra_sb[:])
        nc.sync.dma_start(
            inter_o[:].rearrange("o (p w) -> (o p) w", w=1), inter_sb[:])

        # ---- loss partial ----
        intra_dram = dram.tile([1, PL], F32, tag="intra_d")
        nc.sync.dma_start(
            intra_dram[:].rearrange("o (r g) -> (o r) g", r=NR), intra_sb[:])
        intra32 = epi.tile([PL, 1], F32)
        nc.sync.dma_start(intra32[:],
                          intra_dram[:].rearrange("o (p w) -> (o p) w", w=1))
        t1 = epi.tile([PL, 1], F32)
        # (intra + MARGIN) - inter
        nc.vector.scalar_tensor_tensor(t1[:], intra32[:], MARGIN,
                                       inter_sb[:], ALU.add, ALU.subtract)
        relu = epi.tile([PL, 1], F32)
        nc.vector.tensor_scalar_max(relu[:], t1[:], 0.0)
        loss_ps = ps_misc.tile([1, 1], F32, tag="loss")
        nc.tensor.matmul(loss_ps[:], lhsT=ones32_t[:], rhs=relu[:],
                         start=True, stop=True)
        loss_sb = epi.tile([1, 1], F32)
        nc.scalar.activation(loss_sb[:], loss_ps[:], AF.Copy)
        nc.sync.dma_start(loss_o[:], loss_sb[:])

    nc.compile()
    return nc


def _sqrt_newton(nc, pool, x2, p, w):
    """clip(x2, EPS) -> sqrt with one Newton refinement. Returns [p, w]."""
    xc = pool.tile([p, w], F32, tag=f"nw_xc{p}_{w}")
    nc.vector.tensor_scalar_max(xc[:], x2[:], EPS)
    y0 = pool.tile([p, w], F32, tag=f"nw_y0{p}_{w}")
    nc.scalar.activation(y0[:], xc[:], AF.Sqrt)
    t = pool.tile([p, w], F32, tag=f"nw_t{p}_{w}")
    nc.vector.reciprocal(t[:], y0[:])
    u = pool.tile([p, w], F32, tag=f"nw_u{p}_{w}")
    nc.vector.tensor_tensor(u[:], xc[:], t[:], ALU.mult)
    s = pool.tile([p, w], F32, tag=f"nw_s{p}_{w}")
    nc.vector.tensor_tensor(s[:], y0[:], u[:], ALU.add)
    y1 = pool.tile([p, w], F32, tag=f"nw_y1{p}_{w}")
    nc.vector.tensor_scalar_mul(y1[:], s[:], 0.5)
    return y1


def _make_consts(core):
    ind32 = np.zeros((RW, NR * PL), dtype=np.float32)
    for r_ in range(NR):
        for p_ in range(RW):
            ind32[p_, PL * r_ + GR * r_ + p_ // K] = 1.0
    ident = np.eye(128, dtype=np.float32)
    maskb = np.zeros((PL, P), dtype=np.float32)
    for i in range(PL):
        maskb[i, PL * core + i] = BIG
    ones4 = np.ones((NC_CHUNK, PL), dtype=np.float32)
    ones32 = np.ones((PL, 1), dtype=np.float32)
    return {
        "ind32": ind32, "ident": ident, "maskb": maskb,
        "ones4": ones4, "ones32": ones32,
        "coreid": np.array([[core]], dtype=np.float32),
    }


def kernel(features, targets=None, **unused):
    features = np.ascontiguousarray(np.asarray(features, dtype=np.float32))
    if "nc" not in _CACHE:
        _CACHE["nc"] = _build()
    nc = _CACHE["nc"]

    in_maps = []
    for c in range(N_CORES):
        m = _make_consts(c)
        m["features"] = features[c * R:(c + 1) * R, :]
        in_maps.append(m)

    res = run_bass_kernel_spmd(nc, in_maps, core_ids=list(range(N_CORES)))
    intra = np.concatenate([res.results[c]["intra"][0] for c in range(N_CORES)])
    inter = np.concatenate([res.results[c]["inter"][0] for c in range(N_CORES)])
    loss = np.float32(
        sum(float(res.results[c]["loss"][0, 0]) for c in range(N_CORES)) / P)
    return loss, intra.astype(np.float32), inter.astype(np.float32)


# revision 14
# speedup vs baseline: 1.1667x; 1.1667x over previous
"""ClusterLoss Trainium2 Bass kernel (8-core SPMD).

Problem (P=256 ids, K=16 imgs/id, D=2048):
  centers   = mean over K of features.reshape(P,K,D)           [P, D]
  intra_max = max_k ||f[p,k]-c[p]||  (clipped sqrt)            [P]
  inter_min = min_{j!=i} ||c[i]-c[j]||                         [P]
  loss      = mean(relu(intra_max - inter_min + 10))

Sharding: class dim P across 8 cores (32 classes = 512 rows each).
Per core, over 4 column-chunks (512 f32) x 4 row-groups (128 rows):
  - PE: class-sum matmuls (indicator lhsT) -> centers [32,512] per chunk
  - DMA: broadcast-replicate centers back to [128,512] row alignment
  - DVE: diff = f - rep ; ACT: Square+accumulate -> intra d2
  - centers are PE-transposed and AllGathered per chunk (overlapped),
    each core then computes its [32,256] block of the center Gram to
    get inter_min; cc (center norms) ship with the gather.
Outputs per core: intra [1,32], inter [1,32], loss partial [1,1].
Host: concat + sum/256.
"""

import numpy as np

import concourse.bacc as bacc
import concourse.tile as tile
from concourse import mybir
from concourse.bass_utils import run_bass_kernel_spmd
from contextlib import ExitStack

F32 = mybir.dt.float32
AF = mybir.ActivationFunctionType
ALU = mybir.AluOpType

N_CORES = 8
P = 256
K = 16
D = 2048
PL = P // N_CORES          # 32 local classes
R = PL * K                 # 512 local rows
NC_CHUNK = 4               # column chunks
CW = D // NC_CHUNK         # 512 cols per chunk
NR = 4                     # row groups
RW = R // NR               # 128 rows per group
GR = PL // NR              # 8 classes per row group
MARGIN = 10.0
EPS = 1e-12
BIG = 1e30

_CACHE = {}


def _build():
    nc = bacc.Bacc("TRN2", target_bir_lowering=False, debug=False,
                   num_devices=N_CORES)

    feat = nc.dram_tensor("features", [R, D], F32, kind="ExternalInput").ap()
    ind32 = nc.dram_tensor("ind32", [RW, NR * PL], F32,
                           kind="ExternalInput").ap()
    ind16 = nc.dram_tensor("ind16", [PL, NR * RW], F32,
                           kind="ExternalInput").ap()
    ident = nc.dram_tensor("ident", [128, 128], F32, kind="ExternalInput").ap()
    maskb = nc.dram_tensor("maskb", [PL, P], F32, kind="ExternalInput").ap()
    ones4 = nc.dram_tensor("ones4", [NC_CHUNK, PL], F32,
                           kind="ExternalInput").ap()
    ones32 = nc.dram_tensor("ones32", [PL, 1], F32, kind="ExternalInput").ap()
    coreid = nc.dram_tensor("coreid", [1, 1], F32, kind="ExternalInput").ap()

    intra_o = nc.dram_tensor("intra", [1, PL], F32, kind="ExternalOutput").ap()
    inter_o = nc.dram_tensor("inter", [1, PL], F32, kind="ExternalOutput").ap()
    loss_o = nc.dram_tensor("loss", [1, 1], F32, kind="ExternalOutput").ap()

    core = None  # resolved via maskb input (per-core constant)

    with tile.TileContext(nc) as tc, ExitStack() as ctx:
        consts = ctx.enter_context(tc.tile_pool(name="consts", bufs=1))
        fpool = ctx.enter_context(tc.tile_pool(name="f", bufs=6))
        reppool = ctx.enter_context(tc.tile_pool(name="rep", bufs=4))
        diffpool = ctx.enter_context(tc.tile_pool(name="diff", bufs=3))
        sqpool = ctx.enter_context(tc.tile_pool(name="sq", bufs=2))
        centpool = ctx.enter_context(tc.tile_pool(name="cent", bufs=2))
        ctTpool = ctx.enter_context(tc.tile_pool(name="ctT", bufs=2))
        ctApool = ctx.enter_context(tc.tile_pool(name="ctA", bufs=4))
        small = ctx.enter_context(tc.tile_pool(name="small", bufs=1))
        epi = ctx.enter_context(tc.tile_pool(name="epi", bufs=1))
        ps_cent = ctx.enter_context(
            tc.tile_pool(name="ps_cent", bufs=1, space="PSUM"))
        ps_t = ctx.enter_context(
            tc.tile_pool(name="ps_t", bufs=1, space="PSUM"))
        ps_g = ctx.enter_context(
            tc.tile_pool(name="ps_g", bufs=1, space="PSUM"))
        ps_misc = ctx.enter_context(
            tc.tile_pool(name="ps_misc", bufs=1, space="PSUM"))
        dram = ctx.enter_context(
            tc.tile_pool(name="dram", bufs=1, space="DRAM"))

        # ---- constants ----
        ind32_t = consts.tile([RW, NR * PL], F32)
        nc.sync.dma_start(ind32_t[:], ind32[:])
        ind16_t = consts.tile([PL, NR * RW], F32)
        nc.sync.dma_start(ind16_t[:], ind16[:])
        ident_t = consts.tile([128, 128], F32)
        nc.sync.dma_start(ident_t[:], ident[:])
        maskb_t = consts.tile([PL, P], F32)
        nc.sync.dma_start(maskb_t[:], maskb[:])
        ones4_t = consts.tile([NC_CHUNK, PL], F32)
        nc.sync.dma_start(ones4_t[:], ones4[:])
        ones32_t = consts.tile([PL, 1], F32)
        nc.sync.dma_start(ones32_t[:], ones32[:])

        # persistent accumulators
        d2cols = small.tile([RW, NR * NC_CHUNK], F32)   # (r,c) -> col 4r+c
        d2all = small.tile([RW, NR], F32)
        cc_loc = small.tile([PL, 1], F32)       # local center norms
        cent_full = small.tile([PL, D], F32)    # all centers (means)

        BF16 = mybir.dt.bfloat16
        SHIP_R = 4 * 128 + 2   # 512 center rows + cc_hi + cc_lo (only c=3)
        ship = dram.tile([NC_CHUNK, SHIP_R, PL], BF16, tag="ship")
        g = dram.tile([N_CORES, NC_CHUNK, SHIP_R, PL], BF16, tag="gath")

        # ---- full-row feature loads: 4 x 1MB contiguous DMAs ----
        f_tiles = []
        for r in range(NR):
            ft = fpool.tile([RW, D], F32, tag="f")
            nc.sync.dma_start(ft[:], feat[r * RW:(r + 1) * RW, :])
            f_tiles.append(ft)

        # ---- centers: per column-chunk accumulate over row groups ----
        for c in range(NC_CHUNK):
            cent_ps = ps_cent.tile([PL, CW], F32, tag="cent")
            for r in range(NR):
                nc.tensor.matmul(cent_ps[:],
                                 lhsT=ind32_t[:, PL * r:PL * (r + 1)],
                                 rhs=f_tiles[r][:, c * CW:(c + 1) * CW],
                                 start=(r == 0), stop=(r == NR - 1))
            # extract + scale to means into the full center tile
            nc.scalar.activation(cent_full[:, c * CW:(c + 1) * CW], cent_ps[:],
                                 AF.Copy, scale=1.0 / K)

            # transpose this chunk for shipping (bf16)
            ctT_ps = ps_t.tile([128, 4 * PL], F32, tag="ctT")
            for s in range(4):
                nc.tensor.transpose(
                    ctT_ps[:, s * PL:(s + 1) * PL],
                    cent_full[:, c * CW + s * 128:c * CW + (s + 1) * 128],
                    ident_t[0:PL, 0:PL])
            ctT_sb = ctTpool.tile([128, 4 * PL], BF16, tag="ctT_sb")
            nc.vector.tensor_copy(ctT_sb[:], ctT_ps[:])
            for s in range(4):
                nc.gpsimd.dma_start(ship[c, s * 128:(s + 1) * 128, :],
                                    ctT_sb[:, s * PL:(s + 1) * PL])

        # local center norms (full D, one pass) + bf16 hi/lo for shipping
        sq_cc = sqpool.tile([PL, D], F32, tag="sqcc")
        nc.scalar.activation(sq_cc[:], cent_full[:], AF.Square,
                             accum_out=cc_loc[:])
        cc_hi = epi.tile([PL, 1], BF16, tag="cchi")
        nc.vector.tensor_copy(cc_hi[:], cc_loc[:])
        cc_lo = epi.tile([PL, 1], BF16, tag="cclo")
        nc.vector.tensor_tensor(cc_lo[:], cc_loc[:], cc_hi[:], ALU.subtract)
        nc.gpsimd.dma_start(ship[3, 4 * 128:4 * 128 + 1, :], cc_hi[:])
        nc.gpsimd.dma_start(ship[3, 4 * 128 + 1:4 * 128 + 2, :], cc_lo[:])

        # one AllGather for everything
        nc.gpsimd.collective_compute(
            "AllGather", ALU.bypass,
            replica_groups=[list(range(N_CORES))],
            ins=[ship[:].opt()], outs=[g[:].opt()])

        # ---- intra: replicate centers via PE matmul (PE is idle here;
        # avoids slow broadcast DMAs), diff on DVE, square+accum on ACT ----
        ps_rep = ctx.enter_context(
            tc.tile_pool(name="ps_rep", bufs=2, space="PSUM"))
        for r in range(NR):
            for c in range(NC_CHUNK):
                rep_ps = ps_rep.tile([RW, CW], F32, tag="rep")
                nc.tensor.matmul(
                    rep_ps[:], lhsT=ind16_t[:, r * RW:(r + 1) * RW],
                    rhs=cent_full[:, c * CW:(c + 1) * CW],
                    start=True, stop=True)
                diff = diffpool.tile([RW, CW], F32, tag="diff")
                nc.vector.tensor_tensor(
                    diff[:], f_tiles[r][:, c * CW:(c + 1) * CW], rep_ps[:],
                    ALU.subtract)
                sq = sqpool.tile([RW, CW], F32, tag="sq")
                nc.scalar.activation(
                    sq[:], diff[:], AF.Square,
                    accum_out=d2cols[:, 4 * r + c:4 * r + c + 1])

        # ---- inter: gram over gathered transposed centers (bf16) ----
        g_ps = ps_g.tile([PL, P], F32, tag="G")
        n_mm = 0
        for c in range(NC_CHUNK):
            for s in range(4):
                ctA = ctApool.tile([128, P], BF16, tag="ctA")
                nc.gpsimd.dma_start(
                    ctA[:].rearrange("p (w i) -> p w i", w=N_CORES),
                    g[:, c, s * 128:(s + 1) * 128, :]
                    .rearrange("w r i -> r w i"))
                lloc = ctApool.tile([128, PL], BF16, tag="lloc")
                nc.gpsimd.dma_start(
                    lloc[:], ship[c, s * 128:(s + 1) * 128, :])
                nc.tensor.matmul(g_ps[:], lhsT=lloc[:], rhs=ctA[:],
                                 start=(n_mm == 0), stop=(n_mm == 15))
                n_mm += 1
        # cc of all cores (hi + lo rows from chunk 3 section)
        cch_sb = small.tile([1, P], BF16)
        ccl_sb = small.tile([1, P], BF16)
        nc.gpsimd.dma_start(cch_sb[:], g[:, 3, 4 * 128:4 * 128 + 1, :])
        nc.gpsimd.dma_start(ccl_sb[:], g[:, 3, 4 * 128 + 1:4 * 128 + 2, :])
        ccp_sb = small.tile([1, P], F32)
        nc.vector.tensor_tensor(ccp_sb[:], cch_sb[:], ccl_sb[:], ALU.add)

        # ccrep [PL, P] = sum over chunks of ccp, broadcast to PL partitions
        ccrep_ps = ps_misc.tile([PL, P], F32, tag="ccrep")
        nc.tensor.matmul(ccrep_ps[:], lhsT=ones4_t[0:1, :], rhs=ccp_sb[:],
                         start=True, stop=True)
        in1c = epi.tile([PL, P], F32)
        nc.vector.tensor_tensor(in1c[:], ccrep_ps[:], maskb_t[:], ALU.add)
        H = epi.tile([PL, P], F32)
        nc.vector.scalar_tensor_tensor(H[:], g_ps[:], -2.0, in1c[:],
                                       ALU.mult, ALU.add)
        m2 = epi.tile([PL, 1], F32)
        nc.vector.tensor_reduce(m2[:], H[:], mybir.AxisListType.X, ALU.min)
        inter2 = epi.tile([PL, 1], F32)
        nc.vector.tensor_tensor(inter2[:], m2[:], cc_loc[:], ALU.add)
        inter_sb = _sqrt_newton(nc, epi, inter2, PL, 1)

        # ---- intra: d2 sums -> per-class max ----
        for r in range(NR):
            nc.vector.tensor_reduce(
                d2all[:, r:r + 1],
                d2cols[:, 4 * r:4 * r + 4], mybir.AxisListType.X, ALU.add)
        d2T_ps = ps_misc.tile([NR, RW], F32, tag="d2T")
        nc.tensor.transpose(d2T_ps[:], d2all[:], ident_t[:])
        d2T = epi.tile([NR, RW], F32)
        nc.vector.tensor_copy(d2T[:], d2T_ps[:])
        dmax = epi.tile([NR, GR], F32)
        nc.vector.tensor_reduce(
            dmax[:], d2T[:].rearrange("r (g k) -> r g k", k=K),
            mybir.AxisListType.X, ALU.max)
        intra_sb = _sqrt_newton(nc, epi, dmax, NR, GR)

        # outputs (view the DRAM side to match on-chip layouts)
        nc.sync.dma_start(
            intra_o[:].rearrange("o (r g) -> (o r) g", r=NR), intra_sb[:])
        nc.sync.dma_start(
            inter_o[:].rearrange("o (p w) -> (o p) w", w=1), inter_sb[:])

        # ---- loss partial ----
        intra_dram = dram.tile([1, PL], F32, tag="intra_d")
        nc.sync.dma_start(
            intra_dram[:].rearrange("o (r g) -> (o r) g", r=NR), intra_sb[:])
        intra32 = epi.tile([PL, 1], F32)
        nc.sync.dma_start(intra32[:],
                          intra_dram[:].rearrange("o (p w) -> (o p) w", w=1))
        t1 = epi.tile([PL, 1], F32)
        # (intra + MARGIN) - inter
        nc.vector.scalar_tensor_tensor(t1[:], intra32[:], MARGIN,
                                       inter_sb[:], ALU.add, ALU.subtract)
        relu = epi.tile([PL, 1], F32)
        nc.vector.tensor_scalar_max(relu[:], t1[:], 0.0)
        loss_ps = ps_misc.tile([1, 1], F32, tag="loss")
        nc.tensor.matmul(loss_ps[:], lhsT=ones32_t[:], rhs=relu[:],
                         start=True, stop=True)
        loss_sb = epi.tile([1, 1], F32)
        nc.scalar.activation(loss_sb[:], loss_ps[:], AF.Copy)
        nc.sync.dma_start(loss_o[:], loss_sb[:])

    nc.compile()
    return nc


def _sqrt_newton(nc, pool, x2, p, w):
    """clip(x2, EPS) -> sqrt with one Newton refinement. Returns [p, w]."""
    xc = pool.tile([p, w], F32, tag=f"nw_xc{p}_{w}")
    nc.vector.tensor_scalar_max(xc[:], x2[:], EPS)
    y0 = pool.tile([p, w], F32, tag=f"nw_y0{p}_{w}")
    nc.scalar.activation(y0[:], xc[:], AF.Sqrt)
    t = pool.tile([p, w], F32, tag=f"nw_t{p}_{w}")
    nc.vector.reciprocal(t[:], y0[:])
    u = pool.tile([p, w], F32, tag=f"nw_u{p}_{w}")
    nc.vector.tensor_tensor(u[:], xc[:], t[:], ALU.mult)
    s = pool.tile([p, w], F32, tag=f"nw_s{p}_{w}")
    nc.vector.tensor_tensor(s[:], y0[:], u[:], ALU.add)
    y1 = pool.tile([p, w], F32, tag=f"nw_y1{p}_{w}")
    nc.vector.tensor_scalar_mul(y1[:], s[:], 0.5)
    return y1


def _make_consts(core):
    ind32 = np.zeros((RW, NR * PL), dtype=np.float32)
    for r_ in range(NR):
        for p_ in range(RW):
            ind32[p_, PL * r_ + GR * r_ + p_ // K] = 1.0
    ident = np.eye(128, dtype=np.float32)
    ind16 = np.zeros((PL, NR * RW), dtype=np.float32)
    for r_ in range(NR):
        for q_ in range(RW):
            ind16[GR * r_ + q_ // K, RW * r_ + q_] = 1.0
    maskb = np.zeros((PL, P), dtype=np.float32)
    for i in range(PL):
        maskb[i, PL * core + i] = BIG
    ones4 = np.ones((NC_CHUNK, PL), dtype=np.float32)
    ones32 = np.ones((PL, 1), dtype=np.float32)
    return {
        "ind32": ind32, "ind16": ind16, "ident": ident, "maskb": maskb,
        "ones4": ones4, "ones32": ones32,
        "coreid": np.array([[core]], dtype=np.float32),
    }


def kernel(features, targets=None, **unused):
    features = np.ascontiguousarray(np.asarray(features, dtype=np.float32))
    if "nc" not in _CACHE:
        _CACHE["nc"] = _build()
    nc = _CACHE["nc"]

    in_maps = []
    for c in range(N_CORES):
        m = _make_consts(c)
        m["features"] = features[c * R:(c + 1) * R, :]
        in_maps.append(m)

    res = run_bass_kernel_spmd(nc, in_maps, core_ids=list(range(N_CORES)))
    intra = np.concatenate([res.results[c]["intra"][0] for c in range(N_CORES)])
    inter = np.concatenate([res.results[c]["inter"][0] for c in range(N_CORES)])
    loss = np.float32(
        sum(float(res.results[c]["loss"][0, 0]) for c in range(N_CORES)) / P)
    return loss, intra.astype(np.float32), inter.astype(np.float32)
